# revision 14
# baseline (speedup 1.0000x reference)
"""Trainium2 Bass kernel for CausalLocalBlock.

Reference computation (B=4, N=4096, D=256, W=7, K=15, H=1024):
    mix = causal_conv1d(x, w_mix, left_pad=2W) + b_mix
    h   = layer_norm(x + mix) * g1 + b1
    ff  = gelu(h @ w_ff1 + b_ff1) @ w_ff2 + b_ff2
    out = layer_norm(h + ff) * g2 + b2

Sharding: 8 cores, core c handles batch c//2, sequence half c%2 (2048
tokens) with a 14-token halo passed in from the host (no collectives).

On-chip layout is D-major (features on partitions, tokens on the free
dim).  The conv runs in bf16 (1 PE cycle/row); the FF matmuls run in
fp8e4 with perf_mode=DoubleRow (2 k-tiles per instruction, ~2x); w1/w2
are pre-scaled by 16/32 on the host so fp8 weights stay out of the
denormal range, and the inverse scale rides the mul slot of the gelu /
o-copy.  Residuals and biases fold into extra PE taps (w_mix[14] += I;
diag(g1)*S2 and (b1+b_ff2) taps in ff2; g1 into w_ff1; c1 = b1@w_ff1 +
b_ff1 into the gelu bias).

LayerNorm stats are ones-matmuls on the PE (partition reduction +
broadcast); T = Q - S^2/D uses ACT Square(S*sqrt(1/D)) + one DVE sub;
d = y - mu is formed mu-free via scalar_tensor_tensor ((S * -1/D) + y);
rstd = Exp(-.5*Ln(T + D*eps) + .5*ln(D)) on ACT.  The LN1 apply path
runs in bf16 (errors re-normalized away by LN2); the LN2 apply path
stays fp32 end-to-end (it writes the final output, so its rounding
hits the result 1:1).  PSUM->SBUF copies and the d ops are split
between DVE (half 0) and GpSimd (half 1) to shorten the chains.

Six chunks (128/512/512/448/384/64) keep the DMA head and the LN2
tail short; emission interleaves each conv half-block's 30 matmuls
around the DR matmul groups of earlier chunks so the PE stream never
starves; per-engine emission order tracks data-readiness order since
ACT/DVE are strict FIFO.  Input DMAs are spread over the sync (wmix
do-half 0) / gpsimd (vecs, dg1, wmix do-half 1, w1, w2) / scalar (x)
queues; outputs leave on gpsimd software-DGE.

This walrus build encodes at most ONE sync-wait command per
instruction, so `split_multiwaits` hoists extra waits onto single-wait
NoOps after Tile scheduling.
"""

import copy
import math
import sys

if "/opt/trn_rl_repo" not in sys.path:
    sys.path.insert(0, "/opt/trn_rl_repo")

import ml_dtypes
import numpy as np

import concourse.bass as bass
import concourse.mybir as mybir
import concourse.tile as tile
from concourse.bass_utils import run_bass_kernel_spmd

B, N, D, W = 4, 4096, 256, 7
K = 2 * W + 1
H = 4 * D
EPS = 1e-5
NCORES = 8
TOK = B * N // NCORES          # 2048 tokens per core
HALO = 2 * W                   # 14
DH = D // 128                  # 2 partition halves of D
HJ = H // 128                  # 8 partition tiles of H

# (token_offset, width) per chunk: small head, tiny tail
CHUNKS = [(0, 128), (128, 512), (640, 512), (1152, 512), (1664, 320), (1984, 64)]
NCHUNK = len(CHUNKS)

S1 = 16.0                      # host scale on w1 (fp8 denormal avoidance)
S2 = 32.0                      # host scale on w2 and the dg1/g1 tap

F32 = mybir.dt.float32
BF16 = mybir.dt.bfloat16
F8 = mybir.dt.float8e4
ACTF = mybir.ActivationFunctionType
OP = mybir.AluOpType
DR = mybir.MatmulPerfMode.DoubleRow


def round_bf16(a):
    return np.ascontiguousarray(a, np.float32).astype(ml_dtypes.bfloat16)


def round_fp8(a):
    return np.ascontiguousarray(a, np.float32).astype(ml_dtypes.float8_e4m3)


def split_multiwaits(nc, max_waits=1):
    """This container's walrus encodes at most one sync-wait command per
    instruction; hoist extra waits onto preceding single-wait NoOps."""
    n = 0
    new_module = copy.replace(nc.m, functions=[])
    for function in nc.m.functions:
        new_function = copy.replace(function, blocks=[])
        new_function.set_allocations_from_list(function.allocations)
        for block in function.blocks:
            new_insts = []
            for inst in block.instructions:
                si = inst.sync_info
                if si is not None and len(si.on_wait) > max_waits:
                    waits = list(si.on_wait)
                    for w in waits[:-max_waits]:
                        n += 1
                        nop = mybir.InstNoOp(name=f"WSPLIT-{n}", ins=[], outs=[])
                        nop.engine = inst.engine
                        nop.sync_info = mybir.SyncInfo(on_wait=[w], on_update=[])
                        new_insts.append(nop)
                    inst.sync_info = mybir.SyncInfo(
                        on_wait=waits[-max_waits:], on_update=list(si.on_update)
                    )
                new_insts.append(inst)
            new_function.blocks.append(copy.replace(block, instructions=new_insts))
        new_module.functions.append(new_function)
    nc.m = new_module
    return n


def build_nc():
    nc = bass.Bass()

    xT = nc.declare_dram_parameter("xT", [D, HALO + TOK], BF16, isOutput=False)
    # wmix packed do-major: [p, do, k, di, 128]
    wmix = nc.declare_dram_parameter("wmix", [128, DH * K * DH * 128], BF16, isOutput=False)
    w1 = nc.declare_dram_parameter("w1", [128, HJ * DH * 128], F8, isOutput=False)
    # w2 packed [p, jp, do, jj, 128]
    w2 = nc.declare_dram_parameter("w2", [128, HJ * DH * 128], F8, isOutput=False)
    dg1 = nc.declare_dram_parameter("dg1", [128, DH * 128], BF16, isOutput=False)
    # vecs columns: bmix(2), c1(8), g2(2), b2(2), brow(2)
    vecs = nc.declare_dram_parameter("vecs", [128, 16], F32, isOutput=False)
    outT = nc.declare_dram_parameter("outT", [D, TOK], F32, isOutput=True)

    xT_v = xT.rearrange("(h p) t -> p h t", p=128)
    wmix_v = wmix.rearrange("p (o k a n) -> p o k a n", o=DH, k=K, a=DH)
    outT_v = outT.rearrange("(h p) t -> p h t", p=128)

    inv_d = 1.0 / D

    with tile.TileContext(nc) as tc:
        with tc.tile_pool(name="persist", bufs=1) as pers:
            # ---- input DMAs, spread across queues ----
            x_sb = pers.tile([128, DH, HALO + TOK], BF16)
            x_edges = [0, 142, 654, 1166, 1678, 1998, HALO + TOK]
            x_dmas = [
                lambda e0=e0, e1=e1: nc.scalar.dma_start(
                    out=x_sb[:, :, e0:e1], in_=xT_v[:, :, e0:e1]
                )
                for e0, e1 in zip(x_edges[:-1], x_edges[1:])
            ]
            x_dmas[0](); x_dmas[1]()

            wmix_sb = pers.tile([128, DH, K, DH, 128], BF16)
            G = ((0, 1), (1, 4), (4, 8), (8, 12), (12, K))
            # do0 groups alternate sync/gpsimd; do1 spread over all three
            for g in (0, 2, 4):
                nc.sync.dma_start(out=wmix_sb[:, 0, G[g][0]:G[g][1]],
                                  in_=wmix_v[:, 0, G[g][0]:G[g][1]])
            for g in (1, 3):
                nc.sync.dma_start(out=wmix_sb[:, 1, G[g][0]:G[g][1]],
                                  in_=wmix_v[:, 1, G[g][0]:G[g][1]])

            vecs_sb = pers.tile([128, 16], F32)
            nc.gpsimd.dma_start(out=vecs_sb, in_=vecs[:, :])
            dg1_sb = pers.tile([128, DH, 128], BF16)
            nc.gpsimd.dma_start(
                out=dg1_sb, in_=dg1.rearrange("p (a n) -> p a n", a=DH)
            )
            for g in (1, 3):
                nc.gpsimd.dma_start(out=wmix_sb[:, 0, G[g][0]:G[g][1]],
                                    in_=wmix_v[:, 0, G[g][0]:G[g][1]])
            for g in (0, 2):
                nc.gpsimd.dma_start(out=wmix_sb[:, 1, G[g][0]:G[g][1]],
                                    in_=wmix_v[:, 1, G[g][0]:G[g][1]])
            w1_sb = pers.tile([128, HJ, DH, 128], F8)
            nc.gpsimd.dma_start(
                out=w1_sb, in_=w1.rearrange("p (j a n) -> p j a n", j=HJ, a=DH)
            )
            w2_sb = pers.tile([128, HJ // 2, DH, 2, 128], F8)
            nc.gpsimd.dma_start(
                out=w2_sb,
                in_=w2.rearrange("p (q o j n) -> p q o j n", q=HJ // 2, o=DH, j=2),
            )

            nc.scalar.dma_start(out=wmix_sb[:, 1, 12:K], in_=wmix_v[:, 1, 12:K])
            for f in x_dmas[2:]:
                f()

            bmix_c = vecs_sb[:, 0:2]
            c1_c = vecs_sb[:, 2:10]
            g2_c = vecs_sb[:, 10:12]
            b2_c = vecs_sb[:, 12:14]
            brow_c = vecs_sb[:, 14:16]

            ones_sb = pers.tile([128, 128], BF16)
            nc.vector.memset(ones_sb, 1.0)
            deps_col = pers.tile([128, 1], F32)
            nc.vector.memset(deps_col, float(D) * EPS)
            ebias_col = pers.tile([128, 1], F32)
            nc.vector.memset(ebias_col, 0.5 * math.log(D))

            h_sb = pers.tile([128, DH, TOK], BF16)
            h8_sb = pers.tile([128, DH, TOK], F8)
            o_sb = pers.tile([128, DH, TOK], BF16)

            with (
                tc.tile_pool(name="conv_ps", bufs=3, space="PSUM") as conv_ps,
                tc.tile_pool(name="ff2_ps", bufs=2, space="PSUM") as ff2_ps,
                tc.tile_pool(name="one_ps", bufs=3, space="PSUM") as one_ps,
                tc.tile_pool(name="work", bufs=2) as work,
            ):
                st = [dict() for _ in range(NCHUNK)]

                # ------- stage emitters (PE parts return thunk lists) -------
                def conv_do(c, do):
                    """30 matmul thunks for chunk c, output half do."""
                    off, cw = CHUNKS[c]
                    yps = conv_ps.tile([128, cw], F32, tag="conv", name=f"yps{c}_{do}")
                    st[c].setdefault("yps", {})[do] = yps
                    taps = [(ki, di) for ki in range(K) for di in range(DH)]
                    return [
                        lambda ki=ki, di=di, i=i: nc.tensor.matmul(
                            yps,
                            wmix_sb[:, do, ki, di],
                            x_sb[:, di, off + ki : off + ki + cw],
                            start=(i == 0),
                            stop=(i == 2 * K - 1),
                        )
                        for i, (ki, di) in enumerate(taps)
                    ]

                def ysb_copy(c, do):
                    off, cw = CHUNKS[c]
                    if do == 0:
                        st[c]["ysb"] = work.tile([128, DH, cw], BF16, tag=f"ysb{c % 2}",
                                                 name=f"ysb{c}")
                    nc.vector.tensor_scalar(
                        out=st[c]["ysb"][:, do, :], in0=st[c]["yps"].pop(do),
                        scalar1=bmix_c[:, do : do + 1], scalar2=None, op0=OP.add,
                    )

                def stats(c, src, pfx):
                    """sq, S/Q matmuls, tv, d for layer-norm over `src`.
                    The LN2 variant (pfx="2") keeps d in fp32."""
                    cw = CHUNKS[c][1]
                    sq = work.tile([128, DH, cw], BF16, tag=f"sq{pfx}{c % 2}",
                                   name=f"sq{pfx}_{c}")
                    nc.vector.tensor_mul(sq, src, src)
                    s_ps = one_ps.tile([128, cw], F32, tag="one", name=f"s{pfx}_{c}")
                    q_ps = one_ps.tile([128, cw], F32, tag="one", name=f"q{pfx}_{c}")
                    for a in range(DH):
                        nc.tensor.matmul(s_ps, ones_sb, src[:, a, :],
                                         start=(a == 0), stop=(a == DH - 1))
                    for a in range(DH):
                        nc.tensor.matmul(q_ps, ones_sb, sq[:, a, :],
                                         start=(a == 0), stop=(a == DH - 1))
                    # t1 = (S * sqrt(1/D))^2 on ACT; tv = Q - t1 on DVE
                    t1 = work.tile([128, cw], F32, tag="t1")
                    nc.scalar.activation(t1, s_ps, ACTF.Square,
                                         bias=0.0, scale=math.sqrt(inv_d))
                    tv = work.tile([128, cw], F32, tag=f"tv{pfx}")
                    nc.vector.tensor_sub(tv, q_ps, t1)
                    st[c][pfx + "tv"] = tv
                    ddt = BF16 if pfx == "1" else F32
                    d = work.tile([128, DH, cw], ddt, tag=f"d{pfx}{c % 2}",
                                  name=f"d{pfx}_{c}")
                    for a in range(DH):
                        nc.vector.scalar_tensor_tensor(
                            out=d[:, a, :], in0=s_ps, scalar=-inv_d,
                            in1=src[:, a, :], op0=OP.mult, op1=OP.add,
                        )
                    st[c][pfx + "d"] = d

                def stats1(c):
                    stats(c, st[c]["ysb"], "1")

                def stats2(c):
                    off, cw = CHUNKS[c]
                    stats(c, o_sb[:, :, off : off + cw], "2")

                def rstd(c, pfx):
                    tv = st[c][pfx + "tv"]
                    cw = CHUNKS[c][1]
                    lnv = work.tile([128, cw], F32, tag="lnv")
                    nc.scalar.activation(lnv, tv, ACTF.Ln, bias=deps_col, scale=1.0)
                    rdt = BF16 if pfx == "1" else F32
                    r = work.tile([128, cw], rdt, tag=f"r{pfx}")
                    nc.scalar.activation(r, lnv, ACTF.Exp, bias=ebias_col, scale=-0.5)
                    st[c][pfx + "r"] = r

                def apl1(c):
                    off, cw = CHUNKS[c]
                    d, r = st[c]["1d"], st[c]["1r"]
                    for a in range(DH):
                        nc.vector.tensor_mul(h_sb[:, a, off : off + cw], d[:, a, :], r)
                    nc.scalar.activation(
                        h8_sb[:, :, off : off + cw], h_sb[:, :, off : off + cw],
                        ACTF.Copy, bias=0.0, scale=1.0,
                    )

                def zg(c):
                    """8 x (DR matmul + gelu), PE/ACT ping-pong."""
                    off, cw = CHUNKS[c]
                    gel = work.tile([128, HJ, cw], F8, tag=f"gel{c % 2}", name=f"gel{c}")
                    st[c]["gel"] = gel
                    for j in range(HJ):
                        zps = one_ps.tile([128, cw], F32, tag="one", name=f"z{c}_{j}")
                        nc.tensor.matmul(
                            zps, w1_sb[:, j], h8_sb[:, :, off : off + cw],
                            start=True, stop=True, perf_mode=DR,
                        )
                        nc.scalar.activation(
                            gel[:, j, :], zps, ACTF.Gelu,
                            bias=c1_c[:, j : j + 1], scale=1.0 / S1,
                        )

                def ff2_do(c, do):
                    """dg1 tap + 4 DR matmul thunks for output half do."""
                    off, cw = CHUNKS[c]
                    gel = st[c]["gel"]
                    ops = ff2_ps.tile([128, cw], F32, tag="ff2", name=f"ops{c}_{do}")
                    st[c].setdefault("ops", {})[do] = ops
                    thunks = [
                        lambda: nc.tensor.matmul(
                            ops, dg1_sb[:, do], h_sb[:, do, off : off + cw],
                            start=True, stop=False,
                        )
                    ]
                    thunks += [
                        lambda q=q: nc.tensor.matmul(
                            ops, w2_sb[:, q, do], gel[:, 2 * q : 2 * q + 2, :],
                            start=False, stop=(q == HJ // 2 - 1),
                            perf_mode=DR, skip_group_check=True,
                        )
                        for q in range(HJ // 2)
                    ]
                    return thunks

                def o_copy(c, do):
                    off, cw = CHUNKS[c]
                    nc.vector.tensor_scalar(
                        out=o_sb[:, do, off : off + cw], in0=st[c]["ops"].pop(do),
                        scalar1=1.0 / S2, scalar2=brow_c[:, do : do + 1],
                        op0=OP.mult, op1=OP.add,
                    )

                def apl2(c):
                    off, cw = CHUNKS[c]
                    d, r = st[c]["2d"], st[c]["2r"]
                    out_t = work.tile([128, DH, cw], F32, tag=f"out{c % 2}",
                                      name=f"out{c}")
                    for a in range(DH):
                        u = work.tile([128, cw], F32, tag="u")
                        nc.vector.tensor_mul(u, d[:, a, :], r)
                        nc.vector.tensor_scalar(
                            out=out_t[:, a, :], in0=u,
                            scalar1=g2_c[:, a : a + 1], scalar2=b2_c[:, a : a + 1],
                            op0=OP.mult, op1=OP.add,
                        )
                    nc.gpsimd.dma_start(out=outT_v[:, :, off : off + cw], in_=out_t)

                def run(thunks):
                    for t in thunks:
                        t()

                # ---------------- emission schedule ----------------
                c00 = conv_do(0, 0); c10 = conv_do(1, 0)
                for g in range(5):
                    run(c00[6 * g : 6 * g + 6]); run(c10[6 * g : 6 * g + 6])
                ysb_copy(0, 0); ysb_copy(1, 0)
                c01 = conv_do(0, 1); c11 = conv_do(1, 1)
                for g in range(5):
                    run(c01[6 * g : 6 * g + 6]); run(c11[6 * g : 6 * g + 6])
                ysb_copy(0, 1); ysb_copy(1, 1)
                stats1(0); stats1(1)
                rstd(0, "1"); rstd(1, "1")
                apl1(0); apl1(1)

                c20 = conv_do(2, 0)
                run(c20[:15]); zg(0); run(c20[15:]); ysb_copy(2, 0)
                c21 = conv_do(2, 1)
                run(c21[:15]); zg(1); run(c21[15:]); ysb_copy(2, 1)

                c30 = conv_do(3, 0)
                run(c30[:10])
                run(ff2_do(0, 0)); o_copy(0, 0)
                run(ff2_do(0, 1)); o_copy(0, 1)
                run(c30[10:20])
                stats1(2); stats2(0)
                run(c30[20:]); ysb_copy(3, 0)
                c31 = conv_do(3, 1)
                run(c31[:10])
                run(ff2_do(1, 0)); o_copy(1, 0)
                run(ff2_do(1, 1)); o_copy(1, 1)
                run(c31[10:20])
                stats2(1)
                run(c31[20:]); ysb_copy(3, 1)
                rstd(2, "1"); rstd(0, "2"); rstd(1, "2")
                apl1(2); apl2(0); apl2(1)

                c40 = conv_do(4, 0)
                run(c40[:15]); zg(2); run(c40[15:]); ysb_copy(4, 0)
                c41 = conv_do(4, 1)
                run(c41[:10])
                stats1(3)
                run(c41[10:20])
                run(ff2_do(2, 0)); o_copy(2, 0)
                run(ff2_do(2, 1)); o_copy(2, 1)
                run(c41[20:]); ysb_copy(4, 1)
                stats2(2); stats1(4)
                rstd(3, "1"); rstd(2, "2"); rstd(4, "1")
                apl1(3); apl2(2); apl1(4)

                c50 = conv_do(5, 0)
                run(c50); zg(3); ysb_copy(5, 0)
                c51 = conv_do(5, 1)
                run(c51[:15]); zg(4); run(c51[15:])
                run(ff2_do(3, 0)); o_copy(3, 0)
                run(ff2_do(3, 1)); o_copy(3, 1)
                ysb_copy(5, 1)
                stats2(3); stats1(5)
                rstd(5, "1"); rstd(3, "2")
                apl1(5); apl2(3)
                zg(5)
                run(ff2_do(4, 0)); o_copy(4, 0)
                run(ff2_do(4, 1)); o_copy(4, 1)
                stats2(4)
                run(ff2_do(5, 0)); o_copy(5, 0)
                run(ff2_do(5, 1)); o_copy(5, 1)
                stats2(5)
                rstd(4, "2"); rstd(5, "2")
                apl2(4); apl2(5)

    split_multiwaits(nc)
    return nc


def _pack_inputs(x, w_mix, b_mix, g1, b1, w_ff1, b_ff1, w_ff2, b_ff2, g2, b2):
    """Host-side packing shared by all cores (weights) + per-core shards."""
    f32 = np.float32
    f64 = np.float64
    Wm = np.array(w_mix, dtype=f64).copy()
    Wm[K - 1] += np.eye(D)
    # [p, do, k, di, 128]
    wmix_p = round_bf16(
        Wm.reshape(K, DH, 128, DH, 128).transpose(2, 3, 0, 1, 4).reshape(128, -1)
    )
    W1g = np.array(g1, f64)[:, None] * np.array(w_ff1, f64) * S1
    # [p, j, di, 128]
    w1_p = round_fp8(
        W1g.reshape(DH, 128, HJ, 128).transpose(1, 2, 0, 3).reshape(128, -1)
    )
    # [p, jp, do, jj, 128]
    w2_p = round_fp8(
        (np.array(w_ff2, f64) * S2)
        .reshape(HJ // 2, 2, 128, DH, 128).transpose(2, 0, 3, 1, 4).reshape(128, -1)
    )
    dg1_p = np.zeros((128, DH, 128), f32)
    for a in range(DH):
        dg1_p[np.arange(128), a, np.arange(128)] = (
            np.array(g1, f32)[a * 128 : (a + 1) * 128] * S2
        )
    dg1_p = round_bf16(dg1_p.reshape(128, -1))
    c1 = (np.array(b1, f64) @ np.array(w_ff1, f64) + np.array(b_ff1, f64)).astype(f32)
    vecs_p = np.zeros((128, 16), f32)
    vecs_p[:, 0:2] = np.array(b_mix, f32).reshape(DH, 128).T
    vecs_p[:, 2:10] = c1.reshape(HJ, 128).T
    vecs_p[:, 10:12] = np.array(g2, f32).reshape(DH, 128).T
    vecs_p[:, 12:14] = np.array(b2, f32).reshape(DH, 128).T
    vecs_p[:, 14:16] = (
        (np.array(b1, f64) + np.array(b_ff2, f64)).astype(f32).reshape(DH, 128).T
    )

    shared = {
        "wmix": wmix_p, "w1": w1_p, "w2": w2_p, "dg1": dg1_p,
        "vecs": vecs_p,
    }
    in_maps = []
    x = np.array(x, f32)
    for core in range(NCORES):
        b, half = divmod(core, 2)
        start = half * TOK
        xT_shard = np.zeros((D, HALO + TOK), f32)
        xT_shard[:, HALO:] = x[b, start : start + TOK].T
        if start > 0:
            xT_shard[:, :HALO] = x[b, start - HALO : start].T
        in_maps.append({"xT": round_bf16(xT_shard), **shared})
    return in_maps


_NC_CACHE = None


def _get_nc():
    global _NC_CACHE
    if _NC_CACHE is None:
        _NC_CACHE = build_nc()
    return _NC_CACHE


def run_spmd(in_maps, **kwargs):
    return run_bass_kernel_spmd(_get_nc(), in_maps, core_ids=list(range(NCORES)), **kwargs)


def assemble(results):
    out = np.empty((B, N, D), np.float32)
    for core in range(NCORES):
        b, half = divmod(core, 2)
        start = half * TOK
        out[b, start : start + TOK, :] = (
            np.asarray(results[core]["outT"]).astype(np.float32).T
        )
    return out


def kernel(**inputs):
    res = run_spmd(_pack_inputs(**inputs))
    return assemble(res.results)
